# revision 3
# baseline (speedup 1.0000x reference)
"""Trainium2 Bass kernel for nn_CausalAttention (GNN message passing).

Math (reference):
    pairs[e] = [img[:, src[e]] ; text[:, tgt[e]]]          # B == H == 128
    a[e]     = sigmoid(w2 . relu(W1 @ pairs[e] + b1) + b2) # per-edge gate
    att_img[b, i] = sum_{e: src[e]=i} a[e] * text[b, tgt[e]]
    att_txt[b, t] = sum_{e: tgt[e]=t} a[e] * img[b, src[e]]

Architecture: output-column sharding, fully on-chip one-hot matmul
gathers/scatters. Core c owns att_img[:, Wc] and att_txt[:, Wc],
Wc = [128c, 128c+128). Single fp16 precision throughout (tolerance is
2e-2; fp16 lands ~4e-4), PSUM accumulates in f32.

Per pipe (img shown; txt symmetric with roles swapped):
  - edges with src in Wc, bucketed by w = tgt >> 7 (8 buckets of
    5x128 slots; unused slots are dummies with key -1).
  - phase A (PE): h = relu(UwinT.T @ ohKT + V8[w].T @ ohLT + b1) with
    host-DMA'd one-hots; per-block N=1 matmuls transpose w2.h into
    [e mod 128, block] layout; sigmoid -> a.
  - phase B: ohlo one-hots built on the otherwise-idle GpSimd engine;
    ohKa = (iota==loc)*a built split across DVE/GpSimd; PE scatters
    M_w[lo, loc] += ohlo.T @ ohKa into PSUM.
  - tail: att[:, loc] = sum_w txtT8[w].T @ M_w (8 fp16 matmuls).
Host concatenates the 8 column slices.
"""

import sys

for _p in ("/opt/trn_rl_repo", "/root/.axon_site/_ro/trn_rl_repo"):
    if _p not in sys.path:
        sys.path.insert(0, _p)

import numpy as np

import concourse.bass as bass
import concourse.tile as tile
from concourse import bacc, mybir

P = 128
DIM = 1024
NCORES = 8
NW = 8            # hi buckets
BPW = 5           # blocks per bucket (capacity 640 vs mean 512)
NBLK = NW * BPW   # 40
EC = NBLK * P     # 5120 edge slots per pipeline
BW = BPW * P      # 640 edges per bucket
HALF = EC // 2    # 2560 one-hot cols per DMA chunk

F32 = mybir.dt.float32
F16 = mybir.dt.float16

# one-hot storage/matmul dtype: F16 (safe) or mybir.dt.float8e4 (half DMA,
# mixed-dtype matmul against fp16 stationaries)
OH_DT = F16
OH_NP = mybir.dt.np(OH_DT)

IS_EQ = mybir.AluOpType.is_equal
MULT = mybir.AluOpType.mult

# cpk column layout (fp16 pack)
C_W2 = 0
C_W1I = 1
C_W1X = C_W1I + P
C_IMG = C_W1X + P
C_TXT = C_IMG + DIM
C_IWIN = C_TXT + DIM
C_TWIN = C_IWIN + P
C_TOT = C_TWIN + P          # 2561

# mpk column layout (f32 pack)
M_LOCI = 0
M_LOI = NBLK
M_LOCT = 2 * NBLK
M_LOT = 3 * NBLK
M_B1 = 4 * NBLK
M_B2 = M_B1 + 1
M_TOT = M_B2 + 1            # 162


def _build_program():
    nc = bacc.Bacc(None, target_bir_lowering=False, debug=False)

    cpk = nc.dram_tensor("cpk", [P, C_TOT], F16, kind="ExternalInput")
    tpk = nc.dram_tensor("tpk", [P, 2 * DIM], F16, kind="ExternalInput")
    mpk = nc.dram_tensor("mpk", [P, M_TOT], F32, kind="ExternalInput")
    ohd = {}
    for s in ("i", "t"):
        for k in ("okt", "olt"):
            for h in range(2):
                nm = f"{s}_{k}{h}"
                ohd[nm] = nc.dram_tensor(nm, [P, HALF], OH_DT, kind="ExternalInput")
    out_img = nc.dram_tensor("out_img", [P, P], F32, kind="ExternalOutput")
    out_txt = nc.dram_tensor("out_txt", [P, P], F32, kind="ExternalOutput")

    with tile.TileContext(nc) as tc:
        with (
            tc.tile_pool(name="const", bufs=1) as cp,
            tc.tile_pool(name="work", bufs=4) as wp,
            tc.tile_pool(name="psH", bufs=2, space="PSUM") as psH,
            tc.tile_pool(name="psM", bufs=1, space="PSUM") as psM,
            tc.tile_pool(name="psS", bufs=1, space="PSUM") as psS,
        ):
            cpk_s = cp.tile([P, C_TOT], F16)
            tpk_s = cp.tile([P, 2 * DIM], F16)
            mpk_s = cp.tile([P, M_TOT], F32)
            iota_i = cp.tile([P, P], mybir.dt.int32)
            iota_f = cp.tile([P, P], F32)
            iota16 = cp.tile([P, P], F16)
            U8 = cp.tile([P, NW, P], F16)
            V8 = cp.tile([P, NW, P], F16)
            UwinT = cp.tile([P, P], F16)
            VwinT = cp.tile([P, P], F16)
            ohlo_s = {"i": cp.tile([P, EC], F16, tag="ohlo_i", name="ohlo_i"),
                      "t": cp.tile([P, EC], F16, tag="ohlo_t", name="ohlo_t")}
            oh_s = {}
            for s in ("i", "t"):
                for k in ("okt", "olt"):
                    for h in range(2):
                        nm = f"{s}_{k}{h}"
                        oh_s[nm] = cp.tile([P, HALF], OH_DT, tag=nm, name=nm)

            # ---- DMA issue: sync carries the img-pipe critical path,
            # scalar carries the txt pipe + tail tables ----
            nc.sync.dma_start(mpk_s[:], mpk[:])
            nc.sync.dma_start(cpk_s[:], cpk[:])
            for nm in ("i_okt0", "i_olt0", "i_okt1", "i_olt1"):
                nc.sync.dma_start(oh_s[nm][:], ohd[nm][:])
            for nm in ("t_okt0", "t_olt0", "t_okt1", "t_olt1"):
                nc.scalar.dma_start(oh_s[nm][:], ohd[nm][:])
            nc.scalar.dma_start(tpk_s[:], tpk[:])

            w2_s = cpk_s[:, C_W2 : C_W2 + 1]
            w1i_s = cpk_s[:, C_W1I : C_W1I + P]
            w1x_s = cpk_s[:, C_W1X : C_W1X + P]
            img_s = cpk_s[:, C_IMG : C_IMG + DIM]
            txt_s = cpk_s[:, C_TXT : C_TXT + DIM]
            iwin_s = cpk_s[:, C_IWIN : C_IWIN + P]
            twin_s = cpk_s[:, C_TWIN : C_TWIN + P]
            b1_s = mpk_s[:, M_B1 : M_B1 + 1]
            b2_s = mpk_s[:, M_B2 : M_B2 + 1]

            nc.gpsimd.iota(iota_i[:], pattern=[[1, P]], base=0, channel_multiplier=0)
            nc.vector.tensor_copy(iota_f[:], iota_i[:])
            nc.vector.tensor_copy(iota16[:], iota_f[:])

            # ---- U/V tables: U8[:, w, :] = (img block w).T @ W1i.T ----
            def build_tab(dst, lhsT, rhs, name):
                ps = psH.tile([P, BW], F32, tag="h_ps", name=name)
                nc.tensor.matmul(ps[:, :P], lhsT, rhs, start=True, stop=True)
                nc.scalar.copy(dst, ps[:, :P])

            for w in range(NW):
                build_tab(U8[:, w, :], img_s[:, w * P : (w + 1) * P], w1i_s,
                          f"u{w}")
                build_tab(V8[:, w, :], txt_s[:, w * P : (w + 1) * P], w1x_s,
                          f"v{w}")
            build_tab(UwinT[:], iwin_s, w1i_s, "uw")
            build_tab(VwinT[:], twin_s, w1x_s, "vw")

            # ---- ohlo for img pipe on GpSimd (idle engine) ----
            lo_i = mpk_s[:, M_LOI : M_LOI + NBLK]
            lo_t = mpk_s[:, M_LOT : M_LOT + NBLK]
            for b in range(NBLK):
                nc.gpsimd.tensor_scalar(
                    out=ohlo_s["i"][:, b * P : (b + 1) * P], in0=iota16[:],
                    scalar1=lo_i[:, b : b + 1], scalar2=None, op0=IS_EQ,
                )

            for side, winT, arb8, loc8, t8off, out_d in (
                ("i", UwinT, V8, mpk_s[:, M_LOCI : M_LOCI + NBLK], 0, out_img),
                ("t", VwinT, U8, mpk_s[:, M_LOCT : M_LOCT + NBLK], DIM, out_txt),
            ):
                m_ps0 = psM.tile([P, 4 * P], F32, tag="m0")
                m_ps1 = psM.tile([P, 4 * P], F32, tag="m1")
                m_ps = [m_ps0, m_ps1]
                acc = psS.tile([P, P], F32, tag="acc")
                a_ps = psS.tile([P, NBLK], F32, tag="a_ps")

                # ---- phase A: per-edge gate a ----
                for w in range(NW):
                    e0 = w * BW
                    h_ = 0 if w < 4 else 1
                    c0 = e0 - h_ * HALF
                    ohKT = oh_s[f"{side}_okt{h_}"][:, c0 : c0 + BW]
                    ohLT = oh_s[f"{side}_olt{h_}"][:, c0 : c0 + BW]
                    h_ps = psH.tile([P, BW], F32, tag="h_ps")
                    for mi, (st, oh_) in enumerate(
                        ((winT[:], ohKT), (arb8[:, w, :], ohLT))
                    ):
                        for o, n in ((0, 4 * P), (4 * P, P)):
                            nc.tensor.matmul(
                                h_ps[:, o : o + n], st, oh_[:, o : o + n],
                                start=(mi == 0), stop=(mi == 1),
                            )
                    h_s = wp.tile([P, BW], F16, tag="h_s")
                    nc.scalar.activation(
                        h_s[:], h_ps[:], mybir.ActivationFunctionType.Relu,
                        bias=b1_s,
                    )
                    for j in range(BPW):
                        b = w * BPW + j
                        nc.tensor.matmul(
                            a_ps[:, b : b + 1], h_s[:, j * P : (j + 1) * P],
                            w2_s, start=True, stop=True,
                        )
                a_s = wp.tile([P, NBLK], F32, tag="a_s")
                nc.scalar.activation(
                    a_s[:], a_ps[:], mybir.ActivationFunctionType.Sigmoid,
                    bias=b2_s,
                )

                # ---- phase B: M_w[lo, loc] += ohlo.T @ ((iota==loc)*a) ----
                ohlo_p = ohlo_s[side]
                for b in range(NBLK):
                    w, j = b // BPW, b % BPW
                    eng = nc.vector if b % 2 == 0 else nc.gpsimd
                    ohKa = wp.tile([P, P], F16, tag="ohKa")
                    eng.tensor_scalar(
                        out=ohKa[:], in0=iota16[:],
                        scalar1=loc8[:, b : b + 1], scalar2=a_s[:, b : b + 1],
                        op0=IS_EQ, op1=MULT,
                    )
                    nc.tensor.matmul(
                        m_ps[w // 4][:, (w % 4) * P : (w % 4 + 1) * P],
                        ohlo_p[:, b * P : (b + 1) * P], ohKa[:],
                        start=(j == 0), stop=(j == BPW - 1),
                        skip_group_check=True,
                    )

                # ---- ohlo for txt pipe: queue on GpSimd between pipes ----
                if side == "i":
                    for b in range(NBLK):
                        nc.gpsimd.tensor_scalar(
                            out=ohlo_s["t"][:, b * P : (b + 1) * P],
                            in0=iota16[:], scalar1=lo_t[:, b : b + 1],
                            scalar2=None, op0=IS_EQ,
                        )

                # ---- tail: att[:, loc] = sum_w arbT8[w].T @ M_w ----
                for w in range(NW):
                    m_s = wp.tile([P, P], F16, tag="m_s")
                    nc.scalar.copy(
                        m_s[:], m_ps[w // 4][:, (w % 4) * P : (w % 4 + 1) * P]
                    )
                    nc.tensor.matmul(
                        acc[:], tpk_s[:, t8off + w * P : t8off + (w + 1) * P],
                        m_s[:], start=(w == 0), stop=(w == NW - 1),
                        skip_group_check=True,
                    )
                out_sb = wp.tile([P, P], F32, tag="out_sb")
                nc.vector.tensor_copy(out_sb[:], acc[:])
                nc.sync.dma_start(out_d[:], out_sb[:])

    nc.compile()
    return nc


_PROGRAM = None


def _get_program():
    global _PROGRAM
    if _PROGRAM is None:
        _PROGRAM = _build_program()
    return _PROGRAM


def _pipe_arrays(key, arb, base):
    """key: window-owning endpoint (src for img pipe); arb: other endpoint.
    Returns ohkt, ohlt [P, EC] one-hots and loc8, lo8 [P, NBLK] f32."""
    kloc = key - base                 # 0..127
    w = arb >> 7                      # bucket
    lo = arb & 127
    slots = np.full(EC, -1, np.int64)  # slot -> edge index or -1
    fill = np.zeros(NW, np.int64)
    order = np.argsort(w, kind="stable")
    for ei in order:
        wb = w[ei]
        assert fill[wb] < BW, f"bucket overflow: {fill[wb]}"
        slots[wb * BW + fill[wb]] = ei
        fill[wb] += 1
    klocs = np.full(EC, -1, np.int64)
    los = np.full(EC, -1, np.int64)
    used = slots >= 0
    klocs[used] = kloc[slots[used]]
    los[used] = lo[slots[used]]
    rng = np.arange(P)
    ohkt = np.ascontiguousarray((klocs[None, :] == rng[:, None]).astype(OH_NP))
    ohlt = np.ascontiguousarray((los[None, :] == rng[:, None]).astype(OH_NP))
    # col layout [P, NBLK]: edge slot e at [e % 128, e // 128]
    loc8 = np.ascontiguousarray(klocs.astype(np.float32).reshape(NBLK, P).T)
    lo8 = np.ascontiguousarray(los.astype(np.float32).reshape(NBLK, P).T)
    return ohkt, ohlt, loc8, lo8


def _t8(x16):
    """[b, col] fp16 -> [lo, w*128 + b] with col = 128w + lo."""
    return np.ascontiguousarray(
        x16.T.reshape(NW, P, P).transpose(1, 0, 2).reshape(P, DIM)
    )


def _make_in_maps(img_features, text_features, src, tgt, W1, b1, w2, b2):
    img16 = img_features.astype(np.float16)
    txt16 = text_features.astype(np.float16)
    w1i16 = np.ascontiguousarray(W1[:, :P].T.astype(np.float16))
    w1x16 = np.ascontiguousarray(W1[:, P:].T.astype(np.float16))
    w2c16 = np.ascontiguousarray(w2.astype(np.float16).reshape(P, 1))
    b1c = np.ascontiguousarray(b1.astype(np.float32).reshape(P, 1))
    b2c = np.full((P, 1), np.float32(b2), dtype=np.float32)
    tpk = np.ascontiguousarray(
        np.concatenate([_t8(txt16), _t8(img16)], axis=1))
    src = np.asarray(src).astype(np.int64)
    tgt = np.asarray(tgt).astype(np.int64)

    in_maps = []
    for c in range(NCORES):
        base = c * P
        cpk = np.concatenate(
            [w2c16, w1i16, w1x16, img16, txt16,
             img16[:, base : base + P], txt16[:, base : base + P]], axis=1)
        m = {"cpk": np.ascontiguousarray(cpk), "tpk": tpk}
        mcols = []
        for s, key, arb in (("i", src, tgt), ("t", tgt, src)):
            sel = (key >= base) & (key < base + P)
            ohkt, ohlt, loc8, lo8 = _pipe_arrays(key[sel], arb[sel], base)
            m[f"{s}_okt0"] = np.ascontiguousarray(ohkt[:, :HALF])
            m[f"{s}_okt1"] = np.ascontiguousarray(ohkt[:, HALF:])
            m[f"{s}_olt0"] = np.ascontiguousarray(ohlt[:, :HALF])
            m[f"{s}_olt1"] = np.ascontiguousarray(ohlt[:, HALF:])
            mcols += [loc8, lo8]
        m["mpk"] = np.ascontiguousarray(
            np.concatenate(mcols + [b1c, b2c], axis=1))
        in_maps.append(m)
    return in_maps


def _run(inputs, trace=False):
    from concourse.bass_utils import run_bass_kernel_spmd

    nc = _get_program()
    in_maps = _make_in_maps(**inputs)
    res = run_bass_kernel_spmd(
        nc, in_maps, core_ids=list(range(NCORES)), trace=trace
    )
    att_img = np.concatenate([r["out_img"] for r in res.results], axis=1)
    att_txt = np.concatenate([r["out_txt"] for r in res.results], axis=1)
    return (np.ascontiguousarray(att_img), np.ascontiguousarray(att_txt)), res


def kernel(**inputs):
    out, _ = _run(inputs, trace=False)
    return out


# revision 6
# speedup vs baseline: 5.0930x; 5.0930x over previous
"""Trainium2 Bass kernel for nn_CausalAttention (GNN message passing).

Math (reference):
    pairs[e] = [img[:, src[e]] ; text[:, tgt[e]]]          # B == H == 128
    a[e]     = sigmoid(w2 . relu(W1 @ pairs[e] + b1) + b2) # per-edge gate
    att_img[b, i] = sum_{e: src[e]=i} a[e] * text[b, tgt[e]]
    att_txt[b, t] = sum_{e: tgt[e]=t} a[e] * img[b, src[e]]

Architecture: output-column sharding, on-chip one-hot matmul gathers/
scatters, single fp16 precision (tolerance 2e-2, this lands ~4e-4).
One-hot matrices ship from host as fp8e4 (exact for 0/1, halves DMA);
the PE accepts mixed fp16 x fp8 operands (probed bit-exact).

Core c owns att_img[:, Wc], att_txt[:, Wc], Wc = [128c, 128c+128).
Per pipe (img shown; txt symmetric, roles swapped):
  - edges with src in Wc, bucketed by w = tgt >> 7 (8 buckets x 5
    blocks x 128 slots; dummy slots have key -1 -> all-zero one-hots).
  - phase A (PE): h = relu(UwinT.T @ ohKT + V8[w].T @ ohLT + b1);
    per-block N=1 matmuls transpose w2.h into [e%128, blk] layout;
    sigmoid -> a [128, 40].
  - phase B: ohKa_bucket = ohK_bucket * broadcast(a) (one DVE op per
    bucket); PE scatters M_w[lo, loc] += ohlo.T @ ohKa (PSUM accum).
  - tail: att[:, loc] = sum_w txtT8[w].T @ M_w (8 fp16 matmuls).
Host concatenates the 8 column slices of each output.
"""

import sys

for _p in ("/opt/trn_rl_repo", "/root/.axon_site/_ro/trn_rl_repo"):
    if _p not in sys.path:
        sys.path.insert(0, _p)

import numpy as np

import concourse.bass as bass
import concourse.tile as tile
from concourse import bacc, mybir

P = 128
DIM = 1024
NCORES = 8
NW = 8            # hi buckets
BPW = 5           # blocks per bucket (capacity 640 vs mean 512)
NBLK = NW * BPW   # 40
EC = NBLK * P     # 5120 edge slots per pipeline
BW = BPW * P      # 640 edges per bucket
HALF = EC // 2    # 2560 one-hot cols per DMA chunk

F32 = mybir.dt.float32
F16 = mybir.dt.float16
F8 = mybir.dt.float8e4
OH_NP = mybir.dt.np(F8)

IS_EQ = mybir.AluOpType.is_equal
MULT = mybir.AluOpType.mult

# cpk column layout (fp16 pack)
C_W2 = 0
C_W1I = 1
C_W1X = C_W1I + P
C_IMG = C_W1X + P
C_TXT = C_IMG + DIM
C_IWIN = C_TXT + DIM
C_TWIN = C_IWIN + P
C_TOT = C_TWIN + P          # 2561

# mpk column layout (f32 pack)
M_B1 = 0
M_B2 = 1
M_TOT = 2


def _build_program():
    nc = bacc.Bacc(None, target_bir_lowering=False, debug=False)

    cpk = nc.dram_tensor("cpk", [P, C_TOT], F16, kind="ExternalInput")
    tpk = nc.dram_tensor("tpk", [P, 2 * DIM], F16, kind="ExternalInput")
    mpk = nc.dram_tensor("mpk", [P, M_TOT], F32, kind="ExternalInput")
    ohd = {}
    for s in ("i", "t"):
        for k in ("okt", "olt"):
            for h in range(2):
                nm = f"{s}_{k}{h}"
                ohd[nm] = nc.dram_tensor(nm, [P, HALF], F8, kind="ExternalInput")
        for k in ("olo", "okk"):
            nm = f"{s}_{k}"
            ohd[nm] = nc.dram_tensor(nm, [P, EC], F8, kind="ExternalInput")
    out_img = nc.dram_tensor("out_img", [P, P], F32, kind="ExternalOutput")
    out_txt = nc.dram_tensor("out_txt", [P, P], F32, kind="ExternalOutput")

    with tile.TileContext(nc) as tc:
        with (
            tc.tile_pool(name="const", bufs=1) as cp,
            tc.tile_pool(name="work", bufs=3) as wp,
            tc.tile_pool(name="ka", bufs=3) as kp,
            tc.tile_pool(name="psH", bufs=2, space="PSUM") as psH,
            tc.tile_pool(name="psM", bufs=1, space="PSUM") as psM,
            tc.tile_pool(name="psS", bufs=1, space="PSUM") as psS,
        ):
            cpk_s = cp.tile([P, C_TOT], F16)
            tpk_s = cp.tile([P, 2 * DIM], F16)
            mpk_s = cp.tile([P, M_TOT], F32)
            U8 = cp.tile([P, NW, P], F16)
            V8 = cp.tile([P, NW, P], F16)
            UwinT = cp.tile([P, P], F16)
            VwinT = cp.tile([P, P], F16)
            oh_s = {}
            for s in ("i", "t"):
                for k in ("okt", "olt"):
                    for h in range(2):
                        nm = f"{s}_{k}{h}"
                        oh_s[nm] = cp.tile([P, HALF], F8, tag=nm, name=nm)
                for k in ("olo", "okk"):
                    nm = f"{s}_{k}"
                    oh_s[nm] = cp.tile([P, EC], F8, tag=nm, name=nm)

            # ---- DMA issue. sync queue: img-pipe critical path in need
            # order; scalar queue: txt pipe + tail tables ----
            nc.sync.dma_start(mpk_s[:], mpk[:])
            nc.sync.dma_start(cpk_s[:], cpk[:])
            for nm in ("i_okt0", "i_olt0", "i_okt1", "i_olt1", "i_olo",
                       "i_okk"):
                nc.sync.dma_start(oh_s[nm][:], ohd[nm][:])
            for nm in ("t_okt0", "t_olt0", "t_okt1", "t_olt1", "t_olo",
                       "t_okk"):
                nc.scalar.dma_start(oh_s[nm][:], ohd[nm][:])
            nc.scalar.dma_start(tpk_s[:], tpk[:])

            w2_s = cpk_s[:, C_W2 : C_W2 + 1]
            w1i_s = cpk_s[:, C_W1I : C_W1I + P]
            w1x_s = cpk_s[:, C_W1X : C_W1X + P]
            img_s = cpk_s[:, C_IMG : C_IMG + DIM]
            txt_s = cpk_s[:, C_TXT : C_TXT + DIM]
            iwin_s = cpk_s[:, C_IWIN : C_IWIN + P]
            twin_s = cpk_s[:, C_TWIN : C_TWIN + P]
            b1_s = mpk_s[:, M_B1 : M_B1 + 1]
            b2_s = mpk_s[:, M_B2 : M_B2 + 1]

            # ---- U/V tables: U8[:, w, :] = (img block w).T @ W1i.T ----
            def build_tab(dst, lhsT, rhs, name):
                ps = psH.tile([P, BW], F32, tag="h_ps", name=name)
                nc.tensor.matmul(ps[:, :P], lhsT, rhs, start=True, stop=True)
                nc.scalar.copy(dst, ps[:, :P])

            for w in range(NW):
                build_tab(U8[:, w, :], img_s[:, w * P : (w + 1) * P], w1i_s,
                          f"u{w}")
                build_tab(V8[:, w, :], txt_s[:, w * P : (w + 1) * P], w1x_s,
                          f"v{w}")
            build_tab(UwinT[:], iwin_s, w1i_s, "uw")
            build_tab(VwinT[:], twin_s, w1x_s, "vw")

            for side, winT, arb8, t8off, out_d in (
                ("i", UwinT, V8, 0, out_img),
                ("t", VwinT, U8, DIM, out_txt),
            ):
                m_ps0 = psM.tile([P, 4 * P], F32, tag="m0")
                m_ps1 = psM.tile([P, 4 * P], F32, tag="m1")
                m_ps = [m_ps0, m_ps1]
                acc = psS.tile([P, P], F32, tag="acc")
                a_ps = psS.tile([P, NBLK], F32, tag="a_ps")

                # ---- phase A: per-edge gate a ----
                for w in range(NW):
                    e0 = w * BW
                    h_ = 0 if w < 4 else 1
                    c0 = e0 - h_ * HALF
                    ohKT = oh_s[f"{side}_okt{h_}"][:, c0 : c0 + BW]
                    ohLT = oh_s[f"{side}_olt{h_}"][:, c0 : c0 + BW]
                    h_ps = psH.tile([P, BW], F32, tag="h_ps")
                    for mi, (st, oh_) in enumerate(
                        ((winT[:], ohKT), (arb8[:, w, :], ohLT))
                    ):
                        for o, n in ((0, 4 * P), (4 * P, P)):
                            nc.tensor.matmul(
                                h_ps[:, o : o + n], st, oh_[:, o : o + n],
                                start=(mi == 0), stop=(mi == 1),
                            )
                    h_s = wp.tile([P, BW], F16, tag="h_s")
                    nc.scalar.activation(
                        h_s[:], h_ps[:], mybir.ActivationFunctionType.Relu,
                        bias=b1_s,
                    )
                    for j in range(BPW):
                        b = w * BPW + j
                        nc.tensor.matmul(
                            a_ps[:, b : b + 1], h_s[:, j * P : (j + 1) * P],
                            w2_s, start=True, stop=True,
                        )
                a_s = wp.tile([P, NBLK], F32, tag="a_s")
                nc.scalar.activation(
                    a_s[:], a_ps[:], mybir.ActivationFunctionType.Sigmoid,
                    bias=b2_s,
                )
                # ---- phase B: M_w[lo, loc] += ohlo.T @ (ohK * a) ----
                olo, okk = oh_s[f"{side}_olo"], oh_s[f"{side}_okk"]
                for w in range(NW):
                    e0 = w * BW
                    ohKa = kp.tile([P, BW], F16, tag="ohKa")
                    nc.vector.tensor_tensor(
                        out=ohKa[:].rearrange("p (b l) -> p b l", b=BPW),
                        in0=okk[:, e0 : e0 + BW].rearrange(
                            "p (b l) -> p b l", b=BPW),
                        in1=a_s[:, w * BPW : (w + 1) * BPW].broadcast_to(
                            (P, BPW, P)),
                        op=MULT,
                    )
                    for j in range(BPW):
                        b = w * BPW + j
                        nc.tensor.matmul(
                            m_ps[w // 4][:, (w % 4) * P : (w % 4 + 1) * P],
                            olo[:, b * P : (b + 1) * P],
                            ohKa[:, j * P : (j + 1) * P],
                            start=(j == 0), stop=(j == BPW - 1),
                            skip_group_check=True,
                        )

                # ---- tail: att[:, loc] = sum_w arbT8[w].T @ M_w ----
                for w in range(NW):
                    m_s = wp.tile([P, P], F16, tag="m_s")
                    nc.scalar.copy(
                        m_s[:], m_ps[w // 4][:, (w % 4) * P : (w % 4 + 1) * P]
                    )
                    nc.tensor.matmul(
                        acc[:], tpk_s[:, t8off + w * P : t8off + (w + 1) * P],
                        m_s[:], start=(w == 0), stop=(w == NW - 1),
                        skip_group_check=True,
                    )
                out_sb = wp.tile([P, P], F32, tag="out_sb")
                nc.vector.tensor_copy(out_sb[:], acc[:])
                nc.sync.dma_start(out_d[:], out_sb[:])

    nc.compile()
    return nc


_PROGRAM = None


def _get_program():
    global _PROGRAM
    if _PROGRAM is None:
        _PROGRAM = _build_program()
    return _PROGRAM


def _pipe_arrays(key, arb, base):
    """key: window-owning endpoint (src for img pipe); arb: other endpoint.
    Returns ohkt, ohlt [P, EC] (gather one-hots, [idx, e]) and
    ohlo, ohk [P, EC] (scatter one-hots, [e%128, blk*128+idx])."""
    kloc = key - base                 # 0..127
    w = arb >> 7                      # bucket
    lo = arb & 127
    slots = np.full(EC, -1, np.int64)  # slot -> edge index or -1
    fill = np.zeros(NW, np.int64)
    order = np.argsort(w, kind="stable")
    for ei in order:
        wb = w[ei]
        assert fill[wb] < BW, f"bucket overflow: {fill[wb]}"
        slots[wb * BW + fill[wb]] = ei
        fill[wb] += 1
    klocs = np.full(EC, -1, np.int64)
    los = np.full(EC, -1, np.int64)
    used = slots >= 0
    klocs[used] = kloc[slots[used]]
    los[used] = lo[slots[used]]
    rng = np.arange(P)
    ohkt = np.ascontiguousarray((klocs[None, :] == rng[:, None]).astype(OH_NP))
    ohlt = np.ascontiguousarray((los[None, :] == rng[:, None]).astype(OH_NP))
    # block-diagonal [e, idx] layouts for the scatter matmuls
    lob = los.reshape(NBLK, P).T      # [e%128, blk]
    klb = klocs.reshape(NBLK, P).T
    ohlo = np.zeros((P, NBLK, P), OH_NP)
    ohk = np.zeros((P, NBLK, P), OH_NP)
    eqlo = lob[:, :, None] == rng[None, None, :]
    eqk = klb[:, :, None] == rng[None, None, :]
    ohlo[eqlo] = OH_NP(1.0)
    ohk[eqk] = OH_NP(1.0)
    return (ohkt, ohlt,
            np.ascontiguousarray(ohlo.reshape(P, EC)),
            np.ascontiguousarray(ohk.reshape(P, EC)))


def _t8(x16):
    """[b, col] fp16 -> [lo, w*128 + b] with col = 128w + lo."""
    return np.ascontiguousarray(
        x16.T.reshape(NW, P, P).transpose(1, 0, 2).reshape(P, DIM)
    )


def _make_in_maps(img_features, text_features, src, tgt, W1, b1, w2, b2):
    img16 = img_features.astype(np.float16)
    txt16 = text_features.astype(np.float16)
    w1i16 = np.ascontiguousarray(W1[:, :P].T.astype(np.float16))
    w1x16 = np.ascontiguousarray(W1[:, P:].T.astype(np.float16))
    w2c16 = np.ascontiguousarray(w2.astype(np.float16).reshape(P, 1))
    b1c = np.ascontiguousarray(b1.astype(np.float32).reshape(P, 1))
    b2c = np.full((P, 1), np.float32(b2), dtype=np.float32)
    tpk = np.ascontiguousarray(
        np.concatenate([_t8(txt16), _t8(img16)], axis=1))
    src = np.asarray(src).astype(np.int64)
    tgt = np.asarray(tgt).astype(np.int64)

    in_maps = []
    for c in range(NCORES):
        base = c * P
        cpk = np.concatenate(
            [w2c16, w1i16, w1x16, img16, txt16,
             img16[:, base : base + P], txt16[:, base : base + P]], axis=1)
        m = {"cpk": np.ascontiguousarray(cpk), "tpk": tpk,
             "mpk": np.ascontiguousarray(np.concatenate([b1c, b2c], axis=1))}
        for s, key, arb in (("i", src, tgt), ("t", tgt, src)):
            sel = (key >= base) & (key < base + P)
            ohkt, ohlt, ohlo, ohk = _pipe_arrays(key[sel], arb[sel], base)
            m[f"{s}_okt0"] = np.ascontiguousarray(ohkt[:, :HALF])
            m[f"{s}_okt1"] = np.ascontiguousarray(ohkt[:, HALF:])
            m[f"{s}_olt0"] = np.ascontiguousarray(ohlt[:, :HALF])
            m[f"{s}_olt1"] = np.ascontiguousarray(ohlt[:, HALF:])
            m[f"{s}_olo"] = ohlo
            m[f"{s}_okk"] = ohk
        in_maps.append(m)
    return in_maps


def _run(inputs, trace=False):
    from concourse.bass_utils import run_bass_kernel_spmd

    nc = _get_program()
    in_maps = _make_in_maps(**inputs)
    res = run_bass_kernel_spmd(
        nc, in_maps, core_ids=list(range(NCORES)), trace=trace
    )
    att_img = np.concatenate([r["out_img"] for r in res.results], axis=1)
    att_txt = np.concatenate([r["out_txt"] for r in res.results], axis=1)
    return (np.ascontiguousarray(att_img), np.ascontiguousarray(att_txt)), res


def kernel(**inputs):
    out, _ = _run(inputs, trace=False)
    return out


# revision 11
# speedup vs baseline: 5.8110x; 1.1410x over previous
"""Trainium2 Bass kernel for nn_CausalAttention (GNN message passing).

Math (reference):
    pairs[e] = [img[:, src[e]] ; text[:, tgt[e]]]          # B == H == 128
    a[e]     = sigmoid(w2 . relu(W1 @ pairs[e] + b1) + b2) # per-edge gate
    att_img[b, i] = sum_{e: src[e]=i} a[e] * text[b, tgt[e]]
    att_txt[b, t] = sum_{e: tgt[e]=t} a[e] * img[b, src[e]]

Architecture: output-column sharding, on-chip one-hot matmul gathers/
scatters, single fp16 precision (tolerance 2e-2, this lands ~4e-4).
One-hot matrices ship from host as fp8e4 (exact for 0/1, halves DMA);
the PE accepts mixed fp16 x fp8 operands (probed bit-exact on HW).

Core c owns att_img[:, Wc], att_txt[:, Wc], Wc = [128c, 128c+128).
Per pipe (img shown; txt symmetric, roles swapped):
  - edges with src in Wc, bucketed by w = tgt >> 7 (8 buckets x 5
    blocks x 128 slots; dummy slots have key -1 -> all-zero one-hots).
  - phase A (PE): h = relu(UwinT.T @ ohKT + V8[w].T @ ohLT + b1);
    per-block N=1 matmuls transpose w2.h into [e%128, blk] layout;
    sigmoid -> a [128, 40].
  - phase B: ohKa_bucket = ohK_bucket * broadcast(a) (one DVE op per
    bucket); PE scatters M_w[lo, loc] += ohlo.T @ ohKa (PSUM accum).
  - tail: att[:, loc] = sum_w txtT8[w].T @ M_w (8 fp16 matmuls).
Scheduling: dummy warm-up matmuls ramp the PE p-state during the DMA
window; phase order A(img), A(txt), B(img), B(txt), tails, so phase-B
inputs stream in while phase A computes. DMAs are issued on both HWDGE
queues (sync + scalar) interleaved in need-order.
Host concatenates the 8 column slices of each output.
"""

import sys

for _p in ("/opt/trn_rl_repo", "/root/.axon_site/_ro/trn_rl_repo"):
    if _p not in sys.path:
        sys.path.insert(0, _p)

import numpy as np

import concourse.tile as tile
from concourse import bacc, mybir

P = 128
DIM = 1024
NCORES = 8
NW = 8            # hi buckets
BPW = 5           # blocks per bucket (capacity 640 vs mean 512)
NBLK = NW * BPW   # 40
EC = NBLK * P     # 5120 edge slots per pipeline
BW = BPW * P      # 640 edges per bucket
HALF = EC // 2    # 2560 one-hot cols per DMA chunk
NWARM = 36        # PE p-state warm-up matmuls

F32 = mybir.dt.float32
F16 = mybir.dt.float16
F8 = mybir.dt.float8e4
OH_NP = mybir.dt.np(F8)

MULT = mybir.AluOpType.mult
RELU = mybir.ActivationFunctionType.Relu
SIGM = mybir.ActivationFunctionType.Sigmoid

# cpk column layout (fp16 pack)
C_W2 = 0
C_W1I = 1
C_W1X = C_W1I + P
C_IMG = C_W1X + P
C_TXT = C_IMG + DIM
C_IWIN = C_TXT + DIM
C_TWIN = C_IWIN + P
C_TOT = C_TWIN + P          # 2561


def _build_program():
    nc = bacc.Bacc(None, target_bir_lowering=False, debug=False)

    cpk = nc.dram_tensor("cpk", [P, C_TOT], F16, kind="ExternalInput")
    tpk = nc.dram_tensor("tpk", [P, 2 * DIM], F16, kind="ExternalInput")
    mpk = nc.dram_tensor("mpk", [P, 2], F32, kind="ExternalInput")
    ohd = {}
    for s in ("i", "t"):
        for k in ("okt", "olt"):
            for h in range(2):
                nm = f"{s}_{k}{h}"
                ohd[nm] = nc.dram_tensor(nm, [P, HALF], F8, kind="ExternalInput")
        for k in ("olo", "okk"):
            nm = f"{s}_{k}"
            ohd[nm] = nc.dram_tensor(nm, [P, EC], F8, kind="ExternalInput")
    out_img = nc.dram_tensor("out_img", [P, P], F32, kind="ExternalOutput")
    out_txt = nc.dram_tensor("out_txt", [P, P], F32, kind="ExternalOutput")

    with tile.TileContext(nc) as tc:
        with (
            tc.tile_pool(name="const", bufs=1) as cp,
            tc.tile_pool(name="work", bufs=3) as wp,
            tc.tile_pool(name="ka", bufs=3) as kp,
            tc.tile_pool(name="psH", bufs=2, space="PSUM") as psH,
            tc.tile_pool(name="psM", bufs=1, space="PSUM") as psM,
            tc.tile_pool(name="psS", bufs=1, space="PSUM") as psS,
        ):
            cpk_s = cp.tile([P, C_TOT], F16)
            tpk_s = cp.tile([P, 2 * DIM], F16)
            mpk_s = cp.tile([P, 2], F32)
            warm_s = cp.tile([P, P], F16)
            U8 = cp.tile([P, NW, P], F16)
            V8 = cp.tile([P, NW, P], F16)
            UwinT = cp.tile([P, P], F16)
            VwinT = cp.tile([P, P], F16)
            oh_s = {}
            for s in ("i", "t"):
                for k in ("okt", "olt"):
                    for h in range(2):
                        nm = f"{s}_{k}{h}"
                        oh_s[nm] = cp.tile([P, HALF], F8, tag=nm, name=nm)
                for k in ("olo", "okk"):
                    nm = f"{s}_{k}"
                    oh_s[nm] = cp.tile([P, EC], F8, tag=nm, name=nm)

            # ---- DMA issue, interleaved across both HWDGE queues in
            # need order: cpk -> A(img) -> A(txt) -> B(img) -> B(txt) ----
            nc.sync.dma_start(cpk_s[:], cpk[:])
            nc.sync.dma_start(mpk_s[:], mpk[:])
            for nm in ("i_okt0", "i_okt1", "t_okt0", "t_okt1"):
                nc.sync.dma_start(oh_s[nm][:], ohd[nm][:])
            for nm in ("i_olt0", "i_olt1", "t_olt0", "t_olt1"):
                nc.scalar.dma_start(oh_s[nm][:], ohd[nm][:])
            nc.sync.dma_start(oh_s["i_olo"][:], ohd["i_olo"][:])
            nc.scalar.dma_start(oh_s["i_okk"][:], ohd["i_okk"][:])
            nc.sync.dma_start(oh_s["t_olo"][:], ohd["t_olo"][:])
            nc.scalar.dma_start(oh_s["t_okk"][:], ohd["t_okk"][:])
            nc.scalar.dma_start(tpk_s[:], tpk[:])

            w2_s = cpk_s[:, C_W2 : C_W2 + 1]
            w1i_s = cpk_s[:, C_W1I : C_W1I + P]
            w1x_s = cpk_s[:, C_W1X : C_W1X + P]
            img_s = cpk_s[:, C_IMG : C_IMG + DIM]
            txt_s = cpk_s[:, C_TXT : C_TXT + DIM]
            iwin_s = cpk_s[:, C_IWIN : C_IWIN + P]
            twin_s = cpk_s[:, C_TWIN : C_TWIN + P]
            b1_s = mpk_s[:, 0:1]
            b2_s = mpk_s[:, 1:2]

            # ---- PE p-state warm-up on junk data (no DMA dependency;
            # ramps 0.65 -> 2.4 GHz while inputs stream in) ----
            nc.vector.memset(warm_s[:], 0.0)
            warm_ps = psH.tile([P, BW], F32, tag="h_ps", name="warm_ps")
            for i in range(NWARM):
                nc.tensor.matmul(warm_ps[:, :P], warm_s[:], warm_s[:],
                                 start=True, stop=True, skip_group_check=True)

            # ---- U/V tables: U8[:, w, :] = (img block w).T @ W1i.T ----
            def build_tab(dst, lhsT, rhs, name):
                ps = psH.tile([P, BW], F32, tag="h_ps", name=name)
                nc.tensor.matmul(ps[:, :P], lhsT, rhs, start=True, stop=True)
                nc.vector.tensor_copy(dst, ps[:, :P])

            for w in range(NW):
                build_tab(U8[:, w, :], img_s[:, w * P : (w + 1) * P], w1i_s,
                          f"u{w}")
                build_tab(V8[:, w, :], txt_s[:, w * P : (w + 1) * P], w1x_s,
                          f"v{w}")
            build_tab(UwinT[:], iwin_s, w1i_s, "uw")
            build_tab(VwinT[:], twin_s, w1x_s, "vw")

            sides = (("i", UwinT, V8, 0, out_img),
                     ("t", VwinT, U8, DIM, out_txt))
            a_sb = {}
            # one PSUM bank for both pipes' a accumulators, one for accs
            a_ps2 = psS.tile([P, 2 * NBLK], F32, tag="a_ps2", name="a_ps2")
            acc2 = psS.tile([P, 2 * P], F32, tag="acc2", name="acc2")

            # ---- phase A both pipes: per-edge gate a ----
            for si, (side, winT, arb8, _t8o, _od) in enumerate(sides):
                a_ps = a_ps2[:, si * NBLK : (si + 1) * NBLK]
                for w in range(NW):
                    e0 = w * BW
                    h_ = 0 if w < 4 else 1
                    c0 = e0 - h_ * HALF
                    ohKT = oh_s[f"{side}_okt{h_}"][:, c0 : c0 + BW]
                    ohLT = oh_s[f"{side}_olt{h_}"][:, c0 : c0 + BW]
                    h_ps = psH.tile([P, BW], F32, tag="h_ps")
                    for mi, (st, oh_) in enumerate(
                        ((winT[:], ohKT), (arb8[:, w, :], ohLT))
                    ):
                        for o, n in ((0, 4 * P), (4 * P, P)):
                            nc.tensor.matmul(
                                h_ps[:, o : o + n], st, oh_[:, o : o + n],
                                start=(mi == 0), stop=(mi == 1),
                            )
                    h_s = wp.tile([P, BW], F16, tag="h_s")
                    nc.scalar.activation(h_s[:], h_ps[:], RELU, bias=b1_s)
                    for j in range(BPW):
                        b = w * BPW + j
                        nc.tensor.matmul(
                            a_ps[:, b : b + 1], h_s[:, j * P : (j + 1) * P],
                            w2_s, start=True, stop=True,
                        )
                a_s = wp.tile([P, NBLK], F32, tag=f"a_s_{side}",
                              name=f"a_s_{side}")
                nc.scalar.activation(a_s[:], a_ps[:], SIGM, bias=b2_s)
                a_sb[side] = a_s

            # ---- phase B + tail per pipe (B img, tail img, B txt,
            # tail txt: m_ps banks are reused across pipes) ----
            for si, (side, _w, _a, t8off, out_d) in enumerate(sides):
                m_ps0 = psM.tile([P, 4 * P], F32, tag="m0", name=f"m0{side}")
                m_ps1 = psM.tile([P, 4 * P], F32, tag="m1", name=f"m1{side}")
                m_ps = [m_ps0, m_ps1]
                olo, okk = oh_s[f"{side}_olo"], oh_s[f"{side}_okk"]
                a_s = a_sb[side]
                for w in range(NW):
                    e0 = w * BW
                    ohKa = kp.tile([P, BW], F16, tag="ohKa")
                    nc.vector.tensor_tensor(
                        out=ohKa[:].rearrange("p (b l) -> p b l", b=BPW),
                        in0=okk[:, e0 : e0 + BW].rearrange(
                            "p (b l) -> p b l", b=BPW),
                        in1=a_s[:, w * BPW : (w + 1) * BPW].broadcast_to(
                            (P, BPW, P)),
                        op=MULT,
                    )
                    for j in range(BPW):
                        b = w * BPW + j
                        nc.tensor.matmul(
                            m_ps[w // 4][:, (w % 4) * P : (w % 4 + 1) * P],
                            olo[:, b * P : (b + 1) * P],
                            ohKa[:, j * P : (j + 1) * P],
                            start=(j == 0), stop=(j == BPW - 1),
                            skip_group_check=True,
                        )

                # tail: att[:, loc] = sum_w arbT8[w].T @ M_w
                acc = acc2[:, si * P : (si + 1) * P]
                for w in range(NW):
                    m_s = wp.tile([P, P], F16, tag="m_s")
                    nc.scalar.copy(
                        m_s[:], m_ps[w // 4][:, (w % 4) * P : (w % 4 + 1) * P]
                    )
                    nc.tensor.matmul(
                        acc, tpk_s[:, t8off + w * P : t8off + (w + 1) * P],
                        m_s[:], start=(w == 0), stop=(w == NW - 1),
                        skip_group_check=True,
                    )
                out_sb = wp.tile([P, P], F32, tag="out_sb")
                nc.vector.tensor_copy(out_sb[:], acc)
                nc.sync.dma_start(out_d[:], out_sb[:])

    nc.compile()
    return nc


_PROGRAM = None


def _get_program():
    global _PROGRAM
    if _PROGRAM is None:
        _PROGRAM = _build_program()
    return _PROGRAM


def _pipe_arrays(key, arb, base):
    """key: window-owning endpoint (src for img pipe); arb: other endpoint.
    Returns ohkt, ohlt [P, EC] (gather one-hots, [idx, e]) and
    ohlo, ohk [P, EC] (scatter one-hots, [e%128, blk*128+idx])."""
    kloc = key - base                 # 0..127
    w = arb >> 7                      # bucket
    lo = arb & 127
    slots = np.full(EC, -1, np.int64)  # slot -> edge index or -1
    fill = np.zeros(NW, np.int64)
    order = np.argsort(w, kind="stable")
    for ei in order:
        wb = w[ei]
        assert fill[wb] < BW, f"bucket overflow: {fill[wb]}"
        slots[wb * BW + fill[wb]] = ei
        fill[wb] += 1
    klocs = np.full(EC, -1, np.int64)
    los = np.full(EC, -1, np.int64)
    used = slots >= 0
    klocs[used] = kloc[slots[used]]
    los[used] = lo[slots[used]]
    rng = np.arange(P)
    ohkt = np.ascontiguousarray((klocs[None, :] == rng[:, None]).astype(OH_NP))
    ohlt = np.ascontiguousarray((los[None, :] == rng[:, None]).astype(OH_NP))
    # block-diagonal [e, idx] layouts for the scatter matmuls
    lob = los.reshape(NBLK, P).T      # [e%128, blk]
    klb = klocs.reshape(NBLK, P).T
    ohlo = np.zeros((P, NBLK, P), OH_NP)
    ohk = np.zeros((P, NBLK, P), OH_NP)
    ohlo[lob[:, :, None] == rng[None, None, :]] = OH_NP(1.0)
    ohk[klb[:, :, None] == rng[None, None, :]] = OH_NP(1.0)
    return (ohkt, ohlt,
            np.ascontiguousarray(ohlo.reshape(P, EC)),
            np.ascontiguousarray(ohk.reshape(P, EC)))


def _t8(x16):
    """[b, col] fp16 -> [lo, w*128 + b] with col = 128w + lo."""
    return np.ascontiguousarray(
        x16.T.reshape(NW, P, P).transpose(1, 0, 2).reshape(P, DIM)
    )


def _make_in_maps(img_features, text_features, src, tgt, W1, b1, w2, b2):
    img16 = img_features.astype(np.float16)
    txt16 = text_features.astype(np.float16)
    w1i16 = np.ascontiguousarray(W1[:, :P].T.astype(np.float16))
    w1x16 = np.ascontiguousarray(W1[:, P:].T.astype(np.float16))
    w2c16 = np.ascontiguousarray(w2.astype(np.float16).reshape(P, 1))
    b1c = np.ascontiguousarray(b1.astype(np.float32).reshape(P, 1))
    b2c = np.full((P, 1), np.float32(b2), dtype=np.float32)
    tpk = np.ascontiguousarray(
        np.concatenate([_t8(txt16), _t8(img16)], axis=1))
    src = np.asarray(src).astype(np.int64)
    tgt = np.asarray(tgt).astype(np.int64)

    in_maps = []
    for c in range(NCORES):
        base = c * P
        cpk = np.concatenate(
            [w2c16, w1i16, w1x16, img16, txt16,
             img16[:, base : base + P], txt16[:, base : base + P]], axis=1)
        m = {"cpk": np.ascontiguousarray(cpk), "tpk": tpk,
             "mpk": np.ascontiguousarray(np.concatenate([b1c, b2c], axis=1))}
        for s, key, arb in (("i", src, tgt), ("t", tgt, src)):
            sel = (key >= base) & (key < base + P)
            ohkt, ohlt, ohlo, ohk = _pipe_arrays(key[sel], arb[sel], base)
            m[f"{s}_okt0"] = np.ascontiguousarray(ohkt[:, :HALF])
            m[f"{s}_okt1"] = np.ascontiguousarray(ohkt[:, HALF:])
            m[f"{s}_olt0"] = np.ascontiguousarray(ohlt[:, :HALF])
            m[f"{s}_olt1"] = np.ascontiguousarray(ohlt[:, HALF:])
            m[f"{s}_olo"] = ohlo
            m[f"{s}_okk"] = ohk
        in_maps.append(m)
    return in_maps


def _run(inputs, trace=False):
    from concourse.bass_utils import run_bass_kernel_spmd

    nc = _get_program()
    in_maps = _make_in_maps(**inputs)
    res = run_bass_kernel_spmd(
        nc, in_maps, core_ids=list(range(NCORES)), trace=trace
    )
    att_img = np.concatenate([r["out_img"] for r in res.results], axis=1)
    att_txt = np.concatenate([r["out_txt"] for r in res.results], axis=1)
    return (np.ascontiguousarray(att_img), np.ascontiguousarray(att_txt)), res


def kernel(**inputs):
    out, _ = _run(inputs, trace=False)
    return out
